# revision 22
# baseline (speedup 1.0000x reference)
"""TRN2 Bass kernel for nn_BetweenClusterFC (single-pass fp16, tiled DMA).

Computes out[n] = sum_f (emb_1 @ W1 + b1)[n,f] * (emb_2 @ W2 + b2)[n,f]
for emb_1/emb_2 [32768, 1024] fp32, W [1024, 512], b [512], out [32768] fp32.

Sharding: data-parallel over the 8 NeuronCores — each core handles 4096 rows;
W1/W2 replicated. No cross-core communication; outputs concatenated on the
host.

Strategy:
  - Single fp16 matmul pass per operand (rel err 3.4e-4 vs the 2e-2 gate).
    fp8 is a dead end for ACCURACY, not speed: the error budget math says
    even ONE of the four tensors in single-pass e4m3 gives ~2.4e-2 max rel
    err (> 2e-2 gate; measured 4.8e-2 with all four), and any hi/lo
    correction pass brings fp8-DoubleRow cost back to >= fp16 cost
    (2048-term packed contraction = same cycles as fp16's 1024).  The
    110.4us fp16 matmul phase (512 x 216ns, ~99% peak clock) is the floor.
  - Embeddings are host-retiled to [tile-pair, partition, k-chunk, 256] so
    every DMA reads 4KB contiguous per partition (the naive [D, N] layout
    produced 512B packets, ~150GB/s, starving the PE).  Sustained DMA rate
    observed: ~410 GB/s aggregate on the sync hwdge queue.
  - STARTUP (the main win over the 131.2us baseline): each DMA_DIRECT2D
    trigger costs ~700-840ns ON THE ISSUING ENGINE, and matmuls wait on
    whole-DMA completion semaphores.  The old order (w1 1MB, e1t0 512KB,
    w2, e2t0) put the first real matmul at ~14.9us (7.3us fixed NEFF
    prologue + ~7.5us DMA wait).  Now the four startup tensors are split
    into kc-halves issued in consumption order (w1a e1a w1b e1b w2a e2a
    w2b e2b, 375-768KB each), tiles 0+1 run j-sequential in kc-halves
    interleaved into the delivery window, and the PE warms up on a
    memset scratch tile (no DMA dependency; the old ident16 warmup input
    is gone).  Tiles 2+3 are j-sequential as well (their e2 tile lands
    ~0.8us after e1).  First real matmul ~9.7us, PE near-continuous after.
  - Weights + e-tiles ride the sync hwdge queue in consumption order; only
    the fp32 identity (for the late acc transposes) rides the gpsimd
    queue.  Splitting startup tensors across the scalar/ACT hwdge queue
    was tried twice in the 131us era and regressed (both queues share the
    16 DMA engines; sem/ordering effects delayed the first matmul).
  - Biases are folded out of the device program algebraically:
        out = rowsum(h1*h2) + E1@(W1 b2) + E2@(W2 b1) + b1.b2
    with the rank-1 corrections applied host-side (exactly zero here).
  - Per 128-row tile: two 8-matmul PSUM accumulation groups (kc-interleaved
    for PE pipelining on middle tiles); ACT engine stages h1 PSUM->SBUF
    (DVE cannot read two PSUM operands), then ONE fused DVE
    scalar_tensor_tensor op does the h1*h2 multiply + free-dim reduce into
    acc[:, tile].  NOTE: vector.tensor_tensor_reduce hard-crashes the exec
    unit on this HW (NRT_EXEC_UNIT_UNRECOVERABLE); the InstTensorScalarPtr
    accum_out path works and measures identically.
  - Output transpose is split in three chunks (tiles 0-15, 16-30, 31): the
    first two transpose/copy/store while later tiles' matmuls still run, so
    the end-of-kernel drain is just the last tile's fused reduce plus a
    [128,1] transpose, tiny copy, and 512B store (~2.1us).  DMA cannot
    read PSUM (bass asserts SBUF/DRAM), so the PSUM->SBUF copy stays.
  - Fixed overheads measured from the ntff trace: ~7us NEFF prologue
    (engine barriers, TENSOR_LOAD, const memsets — framework-emitted,
    untouchable) and ~4us teardown (compiler-emitted per-semaphore clears,
    ~170 EVENT_SEMAPHOREs — untouchable from kernel code).

Measured on trn2 (8 cores, SPMD): 130.2 / 131.1 / 131.1 / 130.9 us across
rested runs (prior baseline: 131.2-132.6us).  Run-to-run variance is
+-0.5-0.9us (prologue start jitter 7.0-7.6us + thermal state), larger
than any remaining tunable — sub-1us changes are unverifiable here.  Trace-verified structure: warmups from ~7.0us, first
real matmul ~13.0us (vs 14.9us baseline), matmul phase gapless (only a
~650ns gap at warmup->real handoff and ~875ns before the final acc
transpose), last matmul ends ~125.5us, tail chain (STT 810ns + transpose
+ copy + 512B store completion) ~2.5us, then compiler-emitted teardown.
Startup is DMA-PACKET-latency-bound, not byte-bound: every DMA is 128
per-partition packets; early packets cost ~330ns vs ~157ns steady
(per-engine, 16 engines), so the first (w-chunk + e-chunk) pair = 256
packets ~= 4.6us regardless of chunk size.  Finer chunking (quarters,
kc-singles) and host-side [w|e] blob-combining were modeled: all are
net-zero or worse (more DMAs = more total packets = later last-chunk;
blob halves first mm ~1.3us earlier but adds ~1.5us of later PE idle).
The 8-halves schedule keeps the PE continuously fed from first mm.
NOTE: the device CAN throttle under back-to-back benchmarking (one
session observed +24us on a 3rd consecutive run; a later session saw
none); bench.py (profile-only) on a rested device gives clean numbers.
"""

import sys
import time

import numpy as np

if "/opt/trn_rl_repo" not in sys.path:
    sys.path.insert(0, "/opt/trn_rl_repo")

import concourse.mybir as mybir
import concourse.tile as tile
from concourse import bacc
from concourse.bass_utils import run_bass_kernel_spmd

F32 = mybir.dt.float32
F16 = mybir.dt.float16

N = 32768
D = 1024
F = 512
P = 128
NCORES = 8
R = N // NCORES   # rows per core
RT = R // P       # 128-row tiles per core
KC = D // P       # contraction chunks
TW = 2 * P        # e-tile width (2 row-tiles per DMA)
NT2 = RT // 2     # tile-pairs per core

_CACHE = {}


def _build_program(rows=R, compile=True):
    rt_count = rows // P
    nc = bacc.Bacc("TRN2", target_bir_lowering=False, debug=False)

    def din(name, shape, dt=F16):
        return nc.dram_tensor(name, shape, dt, kind="ExternalInput").ap()

    H = KC // 2
    e1h = din("e1h", [NT2, P, KC, TW])
    e2h = din("e2h", [NT2, P, KC, TW])
    w1h = din("w1h", [P, KC, F])
    w2h = din("w2h", [P, KC, F])
    ident_in = din("ident", [P, P], F32)
    # NOTE: shipping the startup kc-halves as separate HOST-CONTIGUOUS
    # tensors (vs strided slices of w1h/e1h[0]) was tried: w1a/e1a did
    # arrive ~1.3us earlier (243 vs 165 GB/s) and the first real matmul
    # moved 12.96 -> 12.1us, BUT later chunks then serialized (w2a/e2a
    # done ~17/15.8 yet j1a stalled to 19.0) and +3.4us of mid-startup PE
    # gaps appeared: net +2.2us REGRESSION (132.5us).  The strided-slice
    # schedule below is JIT-balanced end-to-end — keep it.
    out = nc.dram_tensor("out", [rows], F32, kind="ExternalOutput").ap()

    mult = mybir.AluOpType.mult

    with tile.TileContext(nc) as tc:
        with (
            tc.tile_pool(name="consts", bufs=1) as consts,
            tc.tile_pool(name="etpool", bufs=16) as etpool,
            tc.tile_pool(name="hpool", bufs=2) as hpool,
            tc.tile_pool(name="fin", bufs=1) as fin_pool,
            tc.tile_pool(name="tp_psum", bufs=1, space="PSUM") as tp_psum,
            tc.tile_pool(name="h_psum", bufs=3, space="PSUM") as h_psum,
        ):
            # gpsimd: scratch memset for PE warmup (no DMA dependency), then
            # the fp32 identity for the late acc transposes on the slow
            # gpsimd queue (needed first at ~tile 15, ~45us in).
            scr = consts.tile([P, P], F16, tag="scr")
            nc.gpsimd.memset(scr[:], 1.0)
            # fp32 ident for the late acc transposes (first read ~45us, at
            # tile 15).  It used to ride the gpsimd queue "out of the way",
            # but its 128 x 512B packets still occupy the SHARED 16 DMA
            # engines at ~10.4-12.5us — stealing ~1.6us of aggregate
            # capacity from the packet-starved early window.  It now goes
            # on the sync queue AFTER the startup halves + first e-pair
            # (lands ~21us, still far ahead of first use).
            ident = consts.tile([P, P], F32, tag="ident")

            # startup DMAs on the sync queue, kc-halves in consumption
            # order: w1a e1a w1b e1b w2a e2a w2b e2b.  Each trigger costs
            # ~700ns on the sync engine; the halves let tile-0/1 matmuls
            # start after ~0.75MB instead of 1.5MB and keep the PE fed
            # while the remaining ~2.25MB stream in at ~410GB/s.
            w1h_sb = consts.tile([P, KC, F], F16, tag="w1h")
            w2h_sb = consts.tile([P, KC, F], F16, tag="w2h")
            ets = [
                etpool.tile([P, KC, TW], F16, tag=f"eth{j}", name=f"eth{j}_t0")
                for j in range(2)
            ]
            for j, (wsb, wh, eh) in enumerate(
                ((w1h_sb, w1h, e1h), (w2h_sb, w2h, e2h))
            ):
                for h in range(2):
                    ksl = slice(h * H, (h + 1) * H)
                    nc.sync.dma_start(wsb[:, ksl, :], wh[:, ksl, :])
                    nc.sync.dma_start(ets[j][:, ksl, :], eh[0][:, ksl, :])

            # warm the PE from ~6.9us (right after the NEFF prologue) so
            # the clock is at max pstate when real matmuls begin ~9.7us.
            # 12 x 216ns spans the window; data can't arrive earlier than
            # ~9.5us (0.75MB after the first ~700ns trigger).
            warm_rhs = scr[:, None, :].to_broadcast((P, 4, P))
            warm_ps = tp_psum.tile([P, 4 * P], F32, tag="warm")
            for _ in range(12):
                nc.tensor.matmul(warm_ps[:], lhsT=scr[:], rhs=warm_rhs,
                                 start=True, stop=True)

            # acc storage in three chunks: tiles 0-15, 16-30, and tile 31
            # alone — so the second transpose/store runs while tile 31's
            # matmuls are still executing and the end-of-kernel drain is just
            # the last tile's fused reduce + a [128,1] transpose + 512B DMA.
            acc_chunks = [(0, 16), (16, 15), (31, 1)]
            accs = [
                fin_pool.tile([P, n], F32, tag=f"acc{ci}", name=f"acc{ci}")
                for ci, (_, n) in enumerate(acc_chunks)
            ]

            ws = (w1h_sb, w2h_sb)

            def tile_epilogue(rt, hps):
                # DVE can read only one PSUM operand per instruction: stage h1
                # through SBUF on the (otherwise idle) ACT engine, then do
                # multiply + free-dim reduce in one fused DVE op.
                h1sb = hpool.tile([P, F], F32, tag="h1sb")
                nc.scalar.copy(h1sb[:], hps[0][:])
                ci = 0 if rt < 16 else (1 if rt < 31 else 2)
                c0, cn = acc_chunks[ci]
                prod = hpool.tile([P, F], F32, tag="prod")
                nc.vector.scalar_tensor_tensor(
                    prod[:], hps[1][:], 1.0, h1sb[:],
                    op0=mult, op1=mult,
                    accum_out=accs[ci][:, rt - c0:rt - c0 + 1],
                )
                # NOTE: f-splitting the last tile's kc7 matmul + reduce into
                # halves (to overlap the 810ns STT with the final matmul)
                # was tried and REGRESSED ~0.4us: the split only buys 108ns
                # of overlap while adding a serial DVE op + 2 sem hops.
                if rt - c0 == cn - 1:
                    # NOTE: scatter-DMAing the final [128,1] acc column
                    # straight to DRAM (128 x 4B descriptors, skipping the
                    # transpose+copy) was tried and REGRESSED ~6us: the
                    # tiny-descriptor DMA's completion semaphores trickle
                    # in ~6us after the data lands, and the NEFF epilogue
                    # waits on them.  Keep the transpose+copy+512B-store.
                    # acc chunk [128 rows-in-tile, cn tiles] -> out[rt*128+p]
                    ps_fin = tp_psum.tile([cn, P], F32, tag="tp",
                                          name=f"ps_fin{ci}")
                    nc.tensor.transpose(ps_fin[:], accs[ci][:], ident[:])
                    fin = fin_pool.tile([cn, P], F32, tag=f"fin{ci}",
                                        name=f"fin{ci}")
                    nc.vector.tensor_copy(fin[:], ps_fin[:])
                    nc.sync.dma_start(
                        out.rearrange("(rt p) -> rt p", p=P)[c0:c0 + cn],
                        fin[:])

            # tiles 0+1: j-sequential in kc-halves, interleaved with the
            # startup DMA arrivals (t0/t1 share the e-tile pair, so each
            # arriving half unlocks 8 matmuls).
            hps01 = [
                [h_psum.tile([P, F], F32, tag=f"h{j}", name=f"hp{j}_t{t}")
                 for j in range(2)]
                for t in range(2)
            ]
            for j in range(2):
                for h in range(2):
                    for t in range(2):
                        for kc in range(h * H, (h + 1) * H):
                            nc.tensor.matmul(
                                hps01[t][j][:],
                                lhsT=ets[j][:, kc, t * P:(t + 1) * P],
                                rhs=ws[j][:, kc, :],
                                start=(kc == 0),
                                stop=(kc == KC - 1),
                            )
            tile_epilogue(0, hps01[0])
            tile_epilogue(1, hps01[1])

            for rt in range(2, rt_count):
                col = (rt % 2) * P
                if rt % 2 == 0:
                    tp = rt // 2
                    for j, eh in enumerate((e1h, e2h)):
                        eth = etpool.tile([P, KC, TW], F16, tag=f"eth{j}")
                        nc.sync.dma_start(eth[:], eh[tp])
                        ets[j] = eth
                    if rt == 2:
                        nc.sync.dma_start(ident[:], ident_in)

                hps = [
                    h_psum.tile([P, F], F32, tag=f"h{j}", name=f"hp{j}")
                    for j in range(2)
                ]
                if rt < 4 or rt == rt_count - 1:
                    # j-sequential: tiles 2/3's e2 tile lands ~0.8us after
                    # e1 during startup; the last tile overlaps the h1
                    # PSUM->SBUF copy with the j=1 group.
                    order = [(kc, 0) for kc in range(KC)]
                    order += [(kc, 1) for kc in range(KC)]
                else:
                    order = [(kc, j) for kc in range(KC) for j in (0, 1)]
                for kc, j in order:
                    nc.tensor.matmul(
                        hps[j][:],
                        lhsT=ets[j][:, kc, col:col + P],
                        rhs=ws[j][:, kc, :],
                        start=(kc == 0),
                        stop=(kc == KC - 1),
                    )
                tile_epilogue(rt, hps)

    if compile:
        nc.compile()
    return nc


def _get_program():
    if "nc" not in _CACHE:
        _CACHE["nc"] = _build_program()
    return _CACHE["nc"]


def _tile_emb(x):
    # [N, D] fp32 -> fp16 tiled [tp_global=128, p=128, kc=8, r=256] with
    # 4KB contiguous per (tp, p): element (tp, p, kc, r) = x[tp*256+r, kc*128+p]
    xh = np.asarray(x, dtype=np.float32).astype(np.float16)
    return np.ascontiguousarray(
        xh.reshape(N // TW, TW, KC, P).transpose(0, 3, 2, 1))


def _tile_w(w):
    # [D, F] fp32 -> fp16 [p=128, kc=8, f=512]
    wh = np.asarray(w, dtype=np.float32).astype(np.float16)
    return np.ascontiguousarray(wh.reshape(KC, P, F).transpose(1, 0, 2))


def make_in_maps(emb_1, emb_2, W1, b1, W2, b2):
    e1t = _tile_emb(emb_1)
    e2t = _tile_emb(emb_2)
    w1h = _tile_w(W1)
    w2h = _tile_w(W2)
    ident = np.eye(P, dtype=np.float32)
    return [
        {
            "e1h": e1t[c * NT2:(c + 1) * NT2],
            "e2h": e2t[c * NT2:(c + 1) * NT2],
            "w1h": w1h, "w2h": w2h, "ident": ident,
        }
        for c in range(NCORES)
    ]


def kernel(emb_1, emb_2, W1, b1, W2, b2, **_unused):
    nc = _get_program()
    in_maps = make_in_maps(emb_1, emb_2, W1, b1, W2, b2)
    last_err = None
    for attempt in range(3):
        try:
            res = run_bass_kernel_spmd(nc, in_maps, list(range(NCORES))).results
            out = np.concatenate([res[c]["out"] for c in range(NCORES)])
            break
        except Exception as e:  # transient NRT/axon failures observed; retry
            last_err = e
            time.sleep(2.0 * (attempt + 1))
    else:
        raise last_err

    # bias terms, folded out of the device program:
    # out += E1 @ (W1 b2) + E2 @ (W2 b1) + b1.b2  (all zero for this problem)
    b1 = np.asarray(b1, dtype=np.float32)
    b2 = np.asarray(b2, dtype=np.float32)
    if b1.any() or b2.any():
        W1 = np.asarray(W1, dtype=np.float32)
        W2 = np.asarray(W2, dtype=np.float32)
        e1 = np.asarray(emb_1, dtype=np.float32)
        e2 = np.asarray(emb_2, dtype=np.float32)
        out = out + e1 @ (W1 @ b2) + e2 @ (W2 @ b1) + float(b1 @ b2)
    return out
